# revision 25
# baseline (speedup 1.0000x reference)
"""Causal single-head attention (B=4, T=4096, E=204, H=64) on 8 NeuronCores.

Sharding: data-parallel over batch (2 cores per batch element); each core
handles the interleaved half of the 128-row query tiles of its batch. For
odd-parity cores the host swaps adjacent 128-row tile pairs of x so that the
causal loop structure (extents + masks) is identical across all 8 cores -->
one SPMD program, balanced work.

v3: all-bf16 PE operands; host-transposed bf16 x^T; causal masks applied
post-exp as cheap 0/1 multiplies on bf16 P (off the S->exp critical path);
attention loop software-pipelined (AV of iteration k issues after S of k+1,
so the in-order PE never stalls on the ACT exp); V_aug transposes
interleaved with the projections to keep PE activity dense (HAM stays warm).

Per-core pipeline:
  DMA x^T (bf16, host-prepped)
  Q^T/K^T/V^T = W^T @ x^T          (PE, contraction over E=128+76 chunks)
  V_aug = [V | 1]                   (PE transpose of V^T + ones columns)
  for kt (k-tiles, k-outer):        S^T = K_tile @ Q^T   (PE -> PSUM fp32)
      P = exp(scale*S^T) (ACT, PSUM->SBUF bf16); mask diag/pad (DVE mul 0/1)
      acc += V_aug^T @ P            (PE; row 64 accumulates softmax denom)
  out = acc^T[:, :64] * (1/acc^T[:, 64])  (PE transpose + DVE recip/mul)
"""

import sys

if "/opt/trn_rl_repo" not in sys.path:
    sys.path.insert(0, "/opt/trn_rl_repo")

import numpy as np

B, T, E, H = 4, 4096, 204, 64
E1 = 128
E2 = E - E1  # 76
NT = T // 128  # 32 k-tiles
NCORES = 8
SCALE = 1.0 / float(np.sqrt(E))

_CACHE = {}


def _build_nc():
    from contextlib import ExitStack

    import concourse.bacc as bacc
    import concourse.bass as bass
    import concourse.mybir as mybir
    import concourse.tile as tile
    from concourse.masks import make_identity

    f32 = mybir.dt.float32
    bf16 = mybir.dt.bfloat16
    Exp = mybir.ActivationFunctionType.Exp

    nc = bacc.Bacc("TRN2", target_bir_lowering=False, debug=False)

    # host supplies x^T (pair-swapped for odd cores), bf16
    xta_d = nc.dram_tensor("xta", [E1, T], bf16, kind="ExternalInput")
    xtb_d = nc.dram_tensor("xtb", [E2, T], bf16, kind="ExternalInput")
    w_d = nc.dram_tensor("w_all", [E, 3 * H], bf16, kind="ExternalInput")
    # 1.0 = keep, 0.0 = mask, applied to the block past the diagonal tile
    pad_d = nc.dram_tensor("pad01", [128, 1], f32, kind="ExternalInput")
    y_d = nc.dram_tensor("y", [T // 2, H], f32, kind="ExternalOutput")

    with tile.TileContext(nc) as tc, ExitStack() as ctx:
        const = ctx.enter_context(tc.tile_pool(name="const", bufs=1))
        big = ctx.enter_context(tc.tile_pool(name="big", bufs=1))
        ppool = ctx.enter_context(tc.tile_pool(name="pp", bufs=4))
        oapool = ctx.enter_context(tc.tile_pool(name="oa", bufs=4))
        ypool = ctx.enter_context(tc.tile_pool(name="yp", bufs=3))
        rpool = ctx.enter_context(tc.tile_pool(name="rp", bufs=3))
        spool = ctx.enter_context(
            tc.tile_pool(name="S", bufs=3, space=bass.MemorySpace.PSUM)
        )
        accpool = ctx.enter_context(
            tc.tile_pool(name="acc", bufs=2, space=bass.MemorySpace.PSUM)
        )

        # ---- DMAs: few, large transfers (queue init is ~1.2us per DMA) ----
        w_a = const.tile([E1, 3 * H], bf16)
        w_b = const.tile([E2, 3 * H], bf16)
        wsb = {
            "wq": (w_a[:, 0:H], w_b[:, 0:H]),
            "wk": (w_a[:, H : 2 * H], w_b[:, H : 2 * H]),
            "wv": (w_a[:, 2 * H : 3 * H], w_b[:, 2 * H : 3 * H]),
        }
        pad_sb = const.tile([128, 1], f32)
        xT_a = big.tile([E1, T], bf16)
        xT_b = big.tile([E2, T], bf16)

        nc.scalar.dma_start(xT_a[:, 2048:T], xta_d[:, 2048:T])
        nc.scalar.dma_start(xT_b[:, 2048:T], xtb_d[:, 2048:T])
        nc.scalar.dma_start(pad_sb[:], pad_d[:])
        nc.gpsimd.dma_start(xT_b[:, 0:2048], xtb_d[:, 0:2048])
        nc.sync.dma_start(w_a[:], w_d[0:E1, :])
        nc.sync.dma_start(w_b[:], w_d[E1:E, :])
        nc.sync.dma_start(xT_a[:, 0:2048], xta_d[:, 0:2048])

        identf = const.tile([128, 128], f32)
        identb = const.tile([128, 128], bf16)
        tri01 = const.tile([128, 128], bf16)
        make_identity(nc, identf[:])
        make_identity(nc, identb[:])
        # tri01[k, q] = 1 if k <= q else 0   (strict lower triangle masked)
        nc.gpsimd.memset(tri01[:], 1.0)
        nc.gpsimd.affine_select(
            out=tri01[:],
            in_=tri01[:],
            compare_op=mybir.AluOpType.is_ge,
            fill=0.0,
            base=0,
            pattern=[[1, 128]],  # iota = -k + q ; keep where >= 0
            channel_multiplier=-1,
        )

        QT = big.tile([H, T], bf16)
        KT = big.tile([H, T], bf16)
        VT = big.tile([H, T], bf16)
        vaug = big.tile([128, NT * (H + 1)], bf16)
        vaug_r = vaug[:].rearrange("p (k c) -> p k c", c=H + 1)
        ones = const.tile([128, NT], bf16)
        nc.vector.memset(ones[:], 1.0)
        nc.vector.tensor_copy(
            vaug_r[:, :, H : H + 1],
            ones[:].rearrange("p (k o) -> p k o", o=1),
        )

        # ---- projections + V_aug transposes, interleaved to keep PE dense.
        # PSUM->SBUF casts split across engines (gpsimd has no PSUM port):
        # Q on scalar/ACT (idle until the attention loop), K on vector,
        # V alternating between the two.
        for t in range(4):
            sl = slice(t * 1024, (t + 1) * 1024)
            for nm, dst in (("wq", QT), ("wk", KT), ("wv", VT)):
                wa, wb = wsb[nm]
                ps = spool.tile([H, 1024], f32, tag="S", name="psproj")
                for h in range(2):
                    hs = slice(h * 512, (h + 1) * 512)
                    xsl = slice(t * 1024 + h * 512, t * 1024 + (h + 1) * 512)
                    nc.tensor.matmul(ps[:, hs], wa, xT_a[:, xsl], start=True, stop=False)
                    nc.tensor.matmul(ps[:, hs], wb, xT_b[:, xsl], start=False, stop=True)
                on_act = nm == "wq" or (nm == "wv" and t % 2 == 0)
                if on_act:
                    nc.scalar.copy(dst[:, sl], ps[:])
                else:
                    nc.vector.tensor_copy(dst[:, sl], ps[:])
            # V_aug for the 8 k-tiles covered by this 1024-col block: PE
            # transposes staged through accpool PSUM (idle until attention),
            # fanned into the strided V_aug layout by DVE.
            for k4 in range(2):
                pvb = accpool.tile([128, 256], bf16, tag="acc", name="pvb")
                for j in range(4):
                    kt = t * 8 + k4 * 4 + j
                    nc.tensor.transpose(
                        pvb[:, j * 64 : (j + 1) * 64],
                        VT[:, kt * 128 : (kt + 1) * 128],
                        identb[0:H, 0:H],
                    )
                k0 = t * 8 + k4 * 4
                nc.vector.tensor_copy(
                    vaug_r[:, k0 : k0 + 4, 0:H],
                    pvb[:].rearrange("p (k c) -> p k c", c=H),
                )

        # ---- attention: chunk-pair outer (2 live accumulators), k-tiles inner.
        # Software-pipelined: AV of iteration kt issues after S/exp of kt+1.
        QT_r = QT[:].rearrange("p (j t) -> p j t", t=256)  # even 128-tiles at [:, j, 0:128]

        deferred = []  # (oa, a) epilogues, all drained after the loop
        ybig = big.tile([128, 16 * H], f32)
        ybig_r = ybig[:].rearrange("p (q c) -> p q c", c=H)

        def drain_epilogue():
            oa, a = deferred.pop(0)
            pf = spool.tile([128, 1024], f32, tag="S")
            pf_r = pf[:, 0 : 4 * (H + 1)].rearrange("p (j c) -> p j c", c=H + 1)
            for j in range(4):
                nc.tensor.transpose(
                    pf_r[:, j, :],
                    oa[:, j * 128 : (j + 1) * 128],
                    identf[0 : H + 1, 0 : H + 1],
                )
            r = rpool.tile([128, 4], f32)
            nc.vector.reciprocal(r[:], pf_r[:, :, H : H + 1])
            for j in range(4):
                nc.vector.tensor_scalar_mul(
                    ybig_r[:, 4 * a + j, :], pf_r[:, j, 0:H], r[:, j : j + 1]
                )

        def emit_av(st):
            kt, pair, v0, vslice, P, acc = st
            am = kt // 8
            for idx, a in enumerate(pair):
                voff = v0 if a == am else 0
                nc.tensor.matmul(
                    acc[a][:, voff * 128 : 512],
                    vslice,
                    P[:, idx * 512 + voff * 128 : (idx + 1) * 512],
                    start=(kt == 0),
                    stop=(kt == 8 * a + 7),
                    skip_group_check=True,
                )
            for a in pair:
                if kt != 8 * a + 7:
                    continue
                oa = oapool.tile([H + 1, 512], f32, name="oa")
                nc.vector.tensor_copy(oa[:], acc[a][:])
                deferred.append((oa, a))

        pending = None
        for chunk_pair in ([0, 1], [2, 3]):
            acc = {
                a: accpool.tile([H + 1, 512], f32, tag="acc", name=f"acc{a}")
                for a in chunk_pair
            }
            ext = 8 * chunk_pair[-1] + 8
            for kt in range(ext):
                am = kt // 8
                pair = [a for a in chunk_pair if a >= am]
                u = kt - 8 * am
                v0 = u // 2
                kslice = KT[:, kt * 128 : (kt + 1) * 128]
                vslice = vaug[:, kt * (H + 1) : (kt + 1) * (H + 1)]
                S = spool.tile([128, 1024], f32, tag="S")
                for idx, a in enumerate(pair):
                    voff = v0 if a == am else 0
                    nc.tensor.matmul(
                        S[:, idx * 512 + voff * 128 : (idx + 1) * 512],
                        kslice,
                        QT_r[:, 4 * a + voff : 4 * a + 4, 0:128],
                        start=True,
                        stop=True,
                    )
                lo = v0 * 128 if pair[0] == am else 0
                hi = len(pair) * 512
                P = ppool.tile([128, 1024], bf16)
                nc.scalar.activation(P[:, lo:hi], S[:, lo:hi], Exp, scale=SCALE)
                if pair[0] == am:
                    blk = P[:, v0 * 128 : v0 * 128 + 128]
                    if u % 2 == 0:
                        nc.vector.tensor_mul(blk, blk, tri01[:])
                    else:
                        nc.vector.tensor_scalar_mul(blk, blk, pad_sb[:])
                if pending is not None:
                    emit_av(pending)
                pending = (kt, pair, v0, vslice, P, acc)
        emit_av(pending)
        while deferred:
            drain_epilogue()
        nc.sync.dma_start(
            y_d[:].rearrange("(q p) c -> p q c", p=128), ybig_r[:, :, :]
        )

    nc.compile()
    return nc


def _get_nc():
    if "nc" not in _CACHE:
        _CACHE["nc"] = _build_nc()
    return _CACHE["nc"]


_PAIR_SWAP = np.arange(NT).reshape(-1, 2)[:, ::-1].reshape(-1)  # [1,0,3,2,...]


def _make_in_maps(x, Wq, Wk, Wv):
    from ml_dtypes import bfloat16

    x = np.asarray(x, dtype=np.float32)
    Wall = np.ascontiguousarray(
        np.concatenate(
            [np.asarray(W, dtype=np.float32) for W in (Wq, Wk, Wv)], axis=1
        ).astype(bfloat16)
    )
    assert x.shape == (B, T, E)
    in_maps = []
    for c in range(NCORES):
        b, s = c // 2, c % 2
        xb = x[b]
        if s == 1:
            xb = xb.reshape(NT, 128, E)[_PAIR_SWAP].reshape(T, E)
        xt = np.ascontiguousarray(xb.T.astype(bfloat16))  # [E, T]
        in_maps.append(
            {
                "xta": np.ascontiguousarray(xt[0:E1]),
                "xtb": np.ascontiguousarray(xt[E1:E]),
                "w_all": Wall,
                "pad01": np.full((128, 1), 0.0 if s == 0 else 1.0, np.float32),
            }
        )
    return in_maps


def _gather(results):
    y = np.empty((B, T, H), dtype=np.float32)
    for c in range(NCORES):
        b, s = c // 2, c % 2
        yl = np.asarray(results[c]["y"]).reshape(T // 256, 128, H)
        yv = y[b].reshape(NT, 128, H)
        yv[2 * np.arange(T // 256) + s] = yl
    return y


def kernel(x, Wq, Wk, Wv, mask=True, **_ignored):
    assert bool(mask), "kernel compiled for causal (mask=True)"
    nc = _get_nc()
    from concourse import bass_utils

    in_maps = _make_in_maps(x, Wq, Wk, Wv)
    res = bass_utils.run_bass_kernel_spmd(nc, in_maps, list(range(NCORES)))
    _CACHE["last_result"] = res
    return _gather(res.results)


if __name__ == "__main__":
    # smoke test with random data
    rng = np.random.default_rng(0)
    x = rng.standard_normal((B, T, E), dtype=np.float32)
    s = 1.0 / np.sqrt(E)
    Wq = (rng.standard_normal((E, H)) * s).astype(np.float32)
    Wk = (rng.standard_normal((E, H)) * s).astype(np.float32)
    Wv = (rng.standard_normal((E, H)) * s).astype(np.float32)
    out = kernel(x, Wq, Wk, Wv, True)
    print("out", out.shape, out.dtype, float(np.abs(out).max()))


# revision 26
# speedup vs baseline: 1.6542x; 1.6542x over previous
"""Causal single-head attention (B=4, T=4096, E=204, H=64) on 8 NeuronCores.

Sharding: data-parallel over batch (2 cores per batch element); each core
handles the interleaved half of the 128-row query tiles of its batch. For
odd-parity cores the host swaps adjacent 128-row tile pairs of x so that the
causal loop structure (extents + masks) is identical across all 8 cores -->
one SPMD program, balanced work.

v3: all-bf16 PE operands; host-transposed bf16 x^T; causal masks applied
post-exp as cheap 0/1 multiplies on bf16 P (off the S->exp critical path);
attention loop software-pipelined (AV of iteration k issues after S of k+1,
so the in-order PE never stalls on the ACT exp); V_aug transposes
interleaved with the projections to keep PE activity dense (HAM stays warm).

Per-core pipeline:
  DMA x^T (bf16, host-prepped)
  Q^T/K^T/V^T = W^T @ x^T          (PE, contraction over E=128+76 chunks)
  V_aug = [V | 1]                   (PE transpose of V^T + ones columns)
  for kt (k-tiles, k-outer):        S^T = K_tile @ Q^T   (PE -> PSUM fp32)
      P = exp(scale*S^T) (ACT, PSUM->SBUF bf16); mask diag/pad (DVE mul 0/1)
      acc += V_aug^T @ P            (PE; row 64 accumulates softmax denom)
  out = acc^T[:, :64] * (1/acc^T[:, 64])  (PE transpose + DVE recip/mul)
"""

import sys

if "/opt/trn_rl_repo" not in sys.path:
    sys.path.insert(0, "/opt/trn_rl_repo")

import numpy as np

B, T, E, H = 4, 4096, 204, 64
E1 = 128
E2 = E - E1  # 76
NT = T // 128  # 32 k-tiles
NCORES = 8
SCALE = 1.0 / float(np.sqrt(E))

_CACHE = {}


def _build_nc():
    from contextlib import ExitStack

    import concourse.bacc as bacc
    import concourse.bass as bass
    import concourse.mybir as mybir
    import concourse.tile as tile
    from concourse.masks import make_identity

    f32 = mybir.dt.float32
    bf16 = mybir.dt.bfloat16
    Exp = mybir.ActivationFunctionType.Exp

    nc = bacc.Bacc("TRN2", target_bir_lowering=False, debug=False)

    # host supplies x^T (pair-swapped for odd cores), bf16
    xta_d = nc.dram_tensor("xta", [E1, T], bf16, kind="ExternalInput")
    xtb_d = nc.dram_tensor("xtb", [E2, T], bf16, kind="ExternalInput")
    w_d = nc.dram_tensor("w_all", [E, 3 * H], bf16, kind="ExternalInput")
    # 1.0 = keep, 0.0 = mask, applied to the block past the diagonal tile
    pad_d = nc.dram_tensor("pad01", [128, 1], f32, kind="ExternalInput")
    y_d = nc.dram_tensor("y", [T // 2, H], f32, kind="ExternalOutput")

    with tile.TileContext(nc) as tc, ExitStack() as ctx:
        const = ctx.enter_context(tc.tile_pool(name="const", bufs=1))
        big = ctx.enter_context(tc.tile_pool(name="big", bufs=1))
        ppool = ctx.enter_context(tc.tile_pool(name="pp", bufs=4))
        oapool = ctx.enter_context(tc.tile_pool(name="oa", bufs=4))
        ypool = ctx.enter_context(tc.tile_pool(name="yp", bufs=3))
        rpool = ctx.enter_context(tc.tile_pool(name="rp", bufs=3))
        spool = ctx.enter_context(
            tc.tile_pool(name="S", bufs=3, space=bass.MemorySpace.PSUM)
        )
        accpool = ctx.enter_context(
            tc.tile_pool(name="acc", bufs=2, space=bass.MemorySpace.PSUM)
        )

        # ---- DMAs: few, large transfers (queue init is ~1.2us per DMA) ----
        w_a = const.tile([E1, 3 * H], bf16)
        w_b = const.tile([E2, 3 * H], bf16)
        wsb = {
            "wq": (w_a[:, 0:H], w_b[:, 0:H]),
            "wk": (w_a[:, H : 2 * H], w_b[:, H : 2 * H]),
            "wv": (w_a[:, 2 * H : 3 * H], w_b[:, 2 * H : 3 * H]),
        }
        pad_sb = const.tile([128, 1], f32)
        xT_a = big.tile([E1, T], bf16)
        xT_b = big.tile([E2, T], bf16)

        nc.scalar.dma_start(w_a[:], w_d[0:E1, :])
        nc.scalar.dma_start(w_b[:], w_d[E1:E, :])
        nc.scalar.dma_start(xT_a[:, 2048:3072], xta_d[:, 2048:3072])
        nc.scalar.dma_start(xT_b[:, 2048:3072], xtb_d[:, 2048:3072])
        nc.scalar.dma_start(xT_a[:, 3072:T], xta_d[:, 3072:T])
        nc.scalar.dma_start(xT_b[:, 3072:T], xtb_d[:, 3072:T])
        nc.scalar.dma_start(pad_sb[:], pad_d[:])
        nc.gpsimd.dma_start(xT_b[:, 0:1024], xtb_d[:, 0:1024])
        nc.gpsimd.dma_start(xT_b[:, 1024:2048], xtb_d[:, 1024:2048])
        nc.sync.dma_start(xT_a[:, 0:1024], xta_d[:, 0:1024])
        nc.sync.dma_start(xT_a[:, 1024:2048], xta_d[:, 1024:2048])

        identf = const.tile([128, 128], f32)
        identb = const.tile([128, 128], bf16)
        tri01 = const.tile([128, 128], bf16)
        make_identity(nc, identf[:])
        make_identity(nc, identb[:])
        # tri01[k, q] = 1 if k <= q else 0   (strict lower triangle masked)
        nc.gpsimd.memset(tri01[:], 1.0)
        nc.gpsimd.affine_select(
            out=tri01[:],
            in_=tri01[:],
            compare_op=mybir.AluOpType.is_ge,
            fill=0.0,
            base=0,
            pattern=[[1, 128]],  # iota = -k + q ; keep where >= 0
            channel_multiplier=-1,
        )

        QT = big.tile([H, T], bf16)
        KT = big.tile([H, T], bf16)
        VT = big.tile([H, T], bf16)
        vaug = big.tile([128, NT * (H + 1)], bf16)
        vaug_r = vaug[:].rearrange("p (k c) -> p k c", c=H + 1)
        ones = const.tile([128, NT], bf16)
        nc.vector.memset(ones[:], 1.0)
        nc.vector.tensor_copy(
            vaug_r[:, :, H : H + 1],
            ones[:].rearrange("p (k o) -> p k o", o=1),
        )

        # ---- projections + V_aug transposes, interleaved to keep PE dense.
        # PSUM->SBUF casts split across engines (gpsimd has no PSUM port):
        # Q on scalar/ACT (idle until the attention loop), K on vector,
        # V alternating between the two.
        for t in range(4):
            sl = slice(t * 1024, (t + 1) * 1024)
            for nm, dst in (("wv", VT), ("wq", QT), ("wk", KT)):
                wa, wb = wsb[nm]
                ps = spool.tile([H, 1024], f32, tag="S", name="psproj")
                for h in range(2):
                    hs = slice(h * 512, (h + 1) * 512)
                    xsl = slice(t * 1024 + h * 512, t * 1024 + (h + 1) * 512)
                    nc.tensor.matmul(ps[:, hs], wa, xT_a[:, xsl], start=True, stop=False)
                    nc.tensor.matmul(ps[:, hs], wb, xT_b[:, xsl], start=False, stop=True)
                if nm == "wv":
                    nc.scalar.copy(dst[:, sl], ps[:])
                else:
                    nc.vector.tensor_copy(dst[:, sl], ps[:])
            # V_aug for the 8 k-tiles covered by this 1024-col block: PE
            # transposes staged through accpool PSUM (idle until attention),
            # fanned into the strided V_aug layout by DVE.
            for k4 in range(2):
                pvb = accpool.tile([128, 256], bf16, tag="acc", name="pvb")
                for j in range(4):
                    kt = t * 8 + k4 * 4 + j
                    nc.tensor.transpose(
                        pvb[:, j * 64 : (j + 1) * 64],
                        VT[:, kt * 128 : (kt + 1) * 128],
                        identb[0:H, 0:H],
                    )
                k0 = t * 8 + k4 * 4
                nc.vector.tensor_copy(
                    vaug_r[:, k0 : k0 + 4, 0:H],
                    pvb[:].rearrange("p (k c) -> p k c", c=H),
                )

        # ---- attention: chunk-pair outer (2 live accumulators), k-tiles inner.
        # Software-pipelined: AV of iteration kt issues after S/exp of kt+1.
        QT_r = QT[:].rearrange("p (j t) -> p j t", t=256)  # even 128-tiles at [:, j, 0:128]

        deferred = []  # (oa, a) epilogues, all drained after the loop
        ybig = big.tile([128, 16 * H], f32)
        ybig_r = ybig[:].rearrange("p (q c) -> p q c", c=H)

        def drain_epilogue():
            oa, a = deferred.pop(0)
            pf = spool.tile([128, 1024], f32, tag="S")
            pf_r = pf[:, 0 : 4 * (H + 1)].rearrange("p (j c) -> p j c", c=H + 1)
            for j in range(4):
                nc.tensor.transpose(
                    pf_r[:, j, :],
                    oa[:, j * 128 : (j + 1) * 128],
                    identf[0 : H + 1, 0 : H + 1],
                )
            r = rpool.tile([128, 4], f32)
            nc.vector.reciprocal(r[:], pf_r[:, :, H : H + 1])
            for j in range(4):
                nc.vector.tensor_scalar_mul(
                    ybig_r[:, 4 * a + j, :], pf_r[:, j, 0:H], r[:, j : j + 1]
                )
            nc.sync.dma_start(
                y_d[4 * a * 128 : (4 * a + 4) * 128, :].rearrange(
                    "(q p) c -> p q c", p=128
                ),
                ybig_r[:, 4 * a : 4 * a + 4, :],
            )

        def emit_av(st):
            kt, pair, v0, vslice, P, acc = st
            am = kt // 8
            for idx, a in enumerate(pair):
                voff = v0 if a == am else 0
                nc.tensor.matmul(
                    acc[a][:, voff * 128 : 512],
                    vslice,
                    P[:, idx * 512 + voff * 128 : (idx + 1) * 512],
                    start=(kt == 0),
                    stop=(kt == 8 * a + 7),
                    skip_group_check=True,
                )
            for a in pair:
                if kt != 8 * a + 7:
                    continue
                oa = oapool.tile([H + 1, 512], f32, name="oa")
                nc.vector.tensor_copy(oa[:], acc[a][:])
                deferred.append((oa, a))

        pending = None
        for chunk_pair in ([0, 1], [2, 3]):
            acc = {
                a: accpool.tile([H + 1, 512], f32, tag="acc", name=f"acc{a}")
                for a in chunk_pair
            }
            ext = 8 * chunk_pair[-1] + 8
            for kt in range(ext):
                am = kt // 8
                pair = [a for a in chunk_pair if a >= am]
                u = kt - 8 * am
                v0 = u // 2
                kslice = KT[:, kt * 128 : (kt + 1) * 128]
                vslice = vaug[:, kt * (H + 1) : (kt + 1) * (H + 1)]
                S = spool.tile([128, 1024], f32, tag="S")
                for idx, a in enumerate(pair):
                    voff = v0 if a == am else 0
                    nc.tensor.matmul(
                        S[:, idx * 512 + voff * 128 : (idx + 1) * 512],
                        kslice,
                        QT_r[:, 4 * a + voff : 4 * a + 4, 0:128],
                        start=True,
                        stop=True,
                    )
                lo = v0 * 128 if pair[0] == am else 0
                hi = len(pair) * 512
                P = ppool.tile([128, 1024], bf16)
                nc.scalar.activation(P[:, lo:hi], S[:, lo:hi], Exp, scale=SCALE)
                if pair[0] == am:
                    blk = P[:, v0 * 128 : v0 * 128 + 128]
                    if u % 2 == 0:
                        nc.vector.tensor_mul(blk, blk, tri01[:])
                    else:
                        nc.vector.tensor_scalar_mul(blk, blk, pad_sb[:])
                if pending is not None:
                    emit_av(pending)
                pending = (kt, pair, v0, vslice, P, acc)
        emit_av(pending)
        while deferred:
            drain_epilogue()

    nc.compile()
    return nc


def _get_nc():
    if "nc" not in _CACHE:
        _CACHE["nc"] = _build_nc()
    return _CACHE["nc"]


_PAIR_SWAP = np.arange(NT).reshape(-1, 2)[:, ::-1].reshape(-1)  # [1,0,3,2,...]


def _make_in_maps(x, Wq, Wk, Wv):
    from ml_dtypes import bfloat16

    x = np.asarray(x, dtype=np.float32)
    Wall = np.ascontiguousarray(
        np.concatenate(
            [np.asarray(W, dtype=np.float32) for W in (Wq, Wk, Wv)], axis=1
        ).astype(bfloat16)
    )
    assert x.shape == (B, T, E)
    in_maps = []
    for c in range(NCORES):
        b, s = c // 2, c % 2
        xb = x[b]
        if s == 1:
            xb = xb.reshape(NT, 128, E)[_PAIR_SWAP].reshape(T, E)
        xt = np.ascontiguousarray(xb.T.astype(bfloat16))  # [E, T]
        in_maps.append(
            {
                "xta": np.ascontiguousarray(xt[0:E1]),
                "xtb": np.ascontiguousarray(xt[E1:E]),
                "w_all": Wall,
                "pad01": np.full((128, 1), 0.0 if s == 0 else 1.0, np.float32),
            }
        )
    return in_maps


def _gather(results):
    y = np.empty((B, T, H), dtype=np.float32)
    for c in range(NCORES):
        b, s = c // 2, c % 2
        yl = np.asarray(results[c]["y"]).reshape(T // 256, 128, H)
        yv = y[b].reshape(NT, 128, H)
        yv[2 * np.arange(T // 256) + s] = yl
    return y


def kernel(x, Wq, Wk, Wv, mask=True, **_ignored):
    assert bool(mask), "kernel compiled for causal (mask=True)"
    nc = _get_nc()
    from concourse import bass_utils

    in_maps = _make_in_maps(x, Wq, Wk, Wv)
    res = bass_utils.run_bass_kernel_spmd(nc, in_maps, list(range(NCORES)))
    _CACHE["last_result"] = res
    return _gather(res.results)


if __name__ == "__main__":
    # smoke test with random data
    rng = np.random.default_rng(0)
    x = rng.standard_normal((B, T, E), dtype=np.float32)
    s = 1.0 / np.sqrt(E)
    Wq = (rng.standard_normal((E, H)) * s).astype(np.float32)
    Wk = (rng.standard_normal((E, H)) * s).astype(np.float32)
    Wv = (rng.standard_normal((E, H)) * s).astype(np.float32)
    out = kernel(x, Wq, Wk, Wv, True)
    print("out", out.shape, out.dtype, float(np.abs(out).max()))
